# revision 32
# baseline (speedup 1.0000x reference)
"""Trainium2 Bass kernel for the nn_LSTMCell problem.

Strategy: data-parallel over the batch dim (4096 -> 8 cores x 512), weights
replicated. All on-chip compute happens in "transposed" orientation
(hidden on PSUM partitions, batch on the free dim) so every matmul operand
can be DMA'd in its natural, contiguous layout:

    gate.T[h, b] = sum_k W.T[k, h] * act.T[k, b]

Matmuls run in fp8e4 (DoubleRow, 2 k-tiles per instruction, 2x bf16
instruction throughput) except the g-gate (tanh path: no sigmoid
attenuation, dominant error contributor), which stays bf16. All weights
(both dtypes) are pre-scaled x256 on the host so the fp8 ones sit in
e4m3's normal range (raw |W|<=0.023 would quantize as subnormals with
~20% relative error); the 1/256 is folded into the scalar-engine
activation instruction: out = func(psum/256 + bias). PSUM accumulation is
fp32; elementwise math is fp32; outputs are written bf16 and upcast on
the host.

The DMA path takes ~10us to move its first bytes and early bandwidth is
the ramp bottleneck, so early bytes are minimized: only bf16 activations
are loaded (x, h) and their fp8 copies are derived on-device by DVE
casts; phase 1 is software-pipelined so the i/f x-side accums of m=0/1
(x + four small fp8 slabs, ~3.75MB) are the only prologue dependencies,
with the bf16 g-gate work deferred into the per-m iterations.

Per core:
  phase 1 iteration m: h-side accums + cf for m, activations,
           c1 = f*c0 + i*tanh(g) (fp32 in SBUF, bf16 out), c1 cast to
           fp8, then i/f x-side accums of m+2.
  phase 2: per h-tile: o gate fp8 matmuls (incl. W_co @ c1.T),
           o = sigmoid(...), h1 = o * tanh(c1), DMA out (bf16).
"""

import numpy as np
import ml_dtypes
from contextlib import ExitStack

BF = ml_dtypes.bfloat16
F8 = ml_dtypes.float8_e4m3   # TRN FP8_EXP4 (max +-240)
W_SCALE = 256.0              # weights pre-scaled into e4m3 normal range

N_CORES = 8
P = 128          # partition dim / k-tile size / m-tile size
BATCH = 4096
IN_DIM = 2048
HID = 2048
B = BATCH // N_CORES          # 512, batch per core = matmul free dim
KI = IN_DIM // P              # 16, k-tiles for x contraction
KH = HID // P                 # 16, k-tiles for h/c contraction
MT = HID // P                 # 16, output h-tiles

W_NAMES = ["ii", "hi", "if_", "hf", "cf", "ic", "hc", "io", "ho", "co"]
X_NAMES = ("ii", "if_", "ic", "io")   # weights contracting over x
# matmuls run in fp8 DoubleRow except the g-gate (tanh path)
FP8_SET = frozenset(W_NAMES) - {"ic", "hc"}


def _build(p, ki, kh, mt, b, fp8_set):
    import concourse.tile as tile
    from concourse import bacc, mybir

    bf16, f32, f8 = mybir.dt.bfloat16, mybir.dt.float32, mybir.dt.float8e4
    Sig = mybir.ActivationFunctionType.Sigmoid
    Tanh = mybir.ActivationFunctionType.Tanh
    DR = mybir.MatmulPerfMode.DoubleRow
    inv_s = 1.0 / W_SCALE

    nc = bacc.Bacc(
        "TRN2",
        target_bir_lowering=False,
        debug=False,
        num_devices=N_CORES,
    )

    def wdt(n):
        return f8 if n in fp8_set else bf16

    need8 = {"x": any(n in fp8_set for n in X_NAMES),
             "h": any(n in fp8_set for n in ("hi", "hf", "hc", "ho"))}

    xTb = nc.dram_tensor("xTb", [p, ki, b], bf16, kind="ExternalInput").ap()
    hTb = nc.dram_tensor("hTb", [p, kh, b], bf16, kind="ExternalInput").ap()
    xT8 = nc.dram_tensor("xT8", [p, ki, b], f8, kind="ExternalInput").ap() if need8["x"] else None
    hT8 = nc.dram_tensor("hT8", [p, kh, b], f8, kind="ExternalInput").ap() if need8["h"] else None
    if "cf" in fp8_set:
        cT = nc.dram_tensor("cT8", [p, kh, b], f8, kind="ExternalInput").ap()
    else:
        cT = nc.dram_tensor("cTb", [p, kh, b], bf16, kind="ExternalInput").ap()
    c0Tb = nc.dram_tensor("c0Tb", [p, mt, b], bf16, kind="ExternalInput").ap()
    bias = nc.dram_tensor("bias", [p, mt, 4], f32, kind="ExternalInput").ap()
    w = {
        n: nc.dram_tensor(
            f"w_{n}", [mt, p, (ki if n in X_NAMES else kh), p],
            wdt(n), kind="ExternalInput",
        ).ap()
        for n in W_NAMES
    }
    ogT = nc.dram_tensor("ogT", [p, mt, b], bf16, kind="ExternalOutput").ap()
    h1T = nc.dram_tensor("h1T", [p, mt, b], bf16, kind="ExternalOutput").ap()
    c1T = nc.dram_tensor("c1T", [p, mt, b], bf16, kind="ExternalOutput").ap()

    with tile.TileContext(nc) as tc, ExitStack() as ctx:
        acts = ctx.enter_context(tc.tile_pool(name="acts", bufs=1))
        wpool = ctx.enter_context(tc.tile_pool(name="w", bufs=2))
        cpool = ctx.enter_context(tc.tile_pool(name="c0", bufs=2))
        tpool = ctx.enter_context(tc.tile_pool(name="temps", bufs=2))
        ppool = ctx.enter_context(tc.tile_pool(name="psum", bufs=8, space="PSUM"))

        act_sb = {}
        act_sb["xb"] = acts.tile([p, ki, b], bf16, tag="xb", name="xb_sb")
        act_sb["hb"] = acts.tile([p, kh, b], bf16, tag="hb", name="hb_sb")
        ckey = "c8" if "cf" in fp8_set else "cb"
        act_sb[ckey] = acts.tile([p, kh, b], f8 if ckey == "c8" else bf16,
                                 tag="cc", name="c_sb")
        if need8["x"]:
            act_sb["x8"] = acts.tile([p, ki, b], f8, tag="x8", name="x8_sb")
        if need8["h"]:
            act_sb["h8"] = acts.tile([p, kh, b], f8, tag="h8", name="h8_sb")

        bias_sb = acts.tile([p, mt, 4], f32, tag="bias", name="bias_sb")
        # staged per-m results of phase 1a, consumed by phase 1b
        i_all = acts.tile([p, mt, b], bf16, tag="i_all", name="i_all")
        t1_all = acts.tile([p, mt, b], bf16, tag="t1_all", name="t1_all")
        c1b_sb = acts.tile([p, mt, b], bf16, tag="c1b", name="c1b_sb")
        c1m_dt = f8 if "co" in fp8_set else bf16
        c1m_sb = acts.tile([p, mt, b], c1m_dt, tag="c1m", name="c1m_sb")

        def load_w(name, tag, m, chunks=1, eng=None, bufs=2):
            nk = w[name].shape[2]
            t = wpool.tile([p, nk, p], wdt(name), tag=tag, name=f"w_{tag}_{m}",
                           bufs=bufs)
            step = max(1, nk // chunks)
            for c in range(0, nk, step):
                (eng or nc.sync).dma_start(t[:, c:c + step], w[name][m, :, c:c + step])
            return t

        def accum(ps, name, w_t, act_key, first, last):
            fp8 = name in fp8_set
            if isinstance(act_key, str):
                a = act_sb[act_key + ("8" if fp8 else "b")]
            else:
                a = act_key
            nk = w_t.shape[1]
            if fp8:
                for t in range(0, nk, 2):
                    nc.tensor.matmul(
                        ps[:], lhsT=w_t[:, t:t + 2, :], rhs=a[:, t:t + 2, :],
                        start=(first and t == 0), stop=(last and t == nk - 2),
                        perf_mode=DR,
                    )
            else:
                for t in range(nk):
                    nc.tensor.matmul(
                        ps[:], lhsT=w_t[:, t], rhs=a[:, t],
                        start=(first and t == 0), stop=(last and t == nk - 1),
                    )

        # ---- phase 1a: i/f gates -- all-fp8 data, minimal ramp bytes ----
        xw = {}       # m -> (w_ii, w_if) tiles
        wg = {}       # m -> w_ic tile (bf16, phase 1b)
        whc = {}      # m -> w_hc tile (bf16, phase 1b)
        pend = {}     # m -> {"i"/"f": psum tile}
        p2w = {}      # m -> (w_io, w_ho, w_co) tiles

        def load_xw(m, eng=None, chunks=1):
            xw[m] = (load_w("ii", "wxi", m, chunks, eng, bufs=3),
                     load_w("if_", "wxf", m, chunks, eng, bufs=3))

        def load_ow(m, eng=None):
            p2w[m] = (load_w("io", "po", m, eng=eng),
                      load_w("ho", "ph", m, eng=eng),
                      load_w("co", "pc", m, eng=eng))

        def x_parts_if(m):
            ps = pend.setdefault(m, {})
            ps["i"] = ppool.tile([p, b], f32, tag="ps", name=f"ps_i{m}")
            accum(ps["i"], "ii", xw[m][0], "x", True, False)
            ps["f"] = ppool.tile([p, b], f32, tag="ps", name=f"ps_f{m}")
            accum(ps["f"], "if_", xw[m][1], "x", True, False)
            del xw[m]

        # prologue: tiny pokes spin up all three DMA queues, then
        # first-use-ordered fp8 loads. The bf16 g-gate data (xb/hb/w_ic/
        # w_hc, ~5MB) is staged lazily through phase 1a's iterations --
        # it isn't needed until phase 1b, ~90us in.
        nc.gpsimd.dma_start(bias_sb[:, 0:8], bias[:, 0:8])
        nc.sync.dma_start(bias_sb[:, 8:12], bias[:, 8:12])
        nc.scalar.dma_start(bias_sb[:, 12:16], bias[:, 12:16])
        w_h = {}
        w_h["hi", 0] = load_w("hi", "w1", 0, chunks=2, eng=nc.sync, bufs=3)
        w_h["hf", 0] = load_w("hf", "w3", 0, chunks=2, eng=nc.sync, bufs=3)
        for lo, hi in ((0, 2), (2, 4), (4, 8)):
            nc.gpsimd.dma_start(act_sb["x8"][:, lo:hi, :], xT8[:, lo:hi, :])
        nc.gpsimd.dma_start(act_sb["h8"][:, 0:4, :], hT8[:, 0:4, :])
        for lo, hi in ((8, 12), (12, ki)):
            nc.gpsimd.dma_start(act_sb["x8"][:, lo:hi, :], xT8[:, lo:hi, :])
        # PE warmup: dummy matmuls on (garbage) SBUF keep the PE busy while
        # the DMA path spins up (~10us to first bytes), so the HAM clock
        # gate reaches 8/8 before the first real matmul instead of running
        # the whole ramp at half clock. Results land in a PSUM bank that is
        # cleared on its next start=True use.
        dum_w = acts.tile([p, 2, p], f8, tag="dum_w", name="dum_w")
        dum_r = acts.tile([p, 2, b], f8, tag="dum_r", name="dum_r")
        nc.vector.memset(dum_w[:], 0)
        nc.vector.memset(dum_r[:], 0)
        warm_ps = ppool.tile([p, b], f32, tag="ps", name="warm_ps")
        for _ in range(20):
            nc.tensor.matmul(warm_ps[:], lhsT=dum_w[:], rhs=dum_r[:],
                             start=True, stop=True, perf_mode=DR)
        load_xw(0, eng=nc.scalar, chunks=2)
        load_xw(1, eng=nc.scalar)
        for c in range(4, kh, 4):
            nc.gpsimd.dma_start(act_sb["h8"][:, c:c + 4, :], hT8[:, c:c + 4, :])
        w_h["cf", 0] = load_w("cf", "w4", 0, eng=nc.sync, bufs=3)
        for c in range(0, kh, 4):
            nc.gpsimd.dma_start(act_sb[ckey][:, c:c + 4, :], cT[:, c:c + 4, :])
        for n, tag in (("hi", "w1"), ("hf", "w3"), ("cf", "w4")):
            w_h[n, 1] = load_w(n, tag, 1, chunks=2, eng=nc.sync, bufs=3)
        load_xw(2)
        for m in (0, 1):
            ps = pend.setdefault(m, {})
            ps["i"] = ppool.tile([p, b], f32, tag="ps", name=f"ps_i{m}")
            ps["f"] = ppool.tile([p, b], f32, tag="ps", name=f"ps_f{m}")
        for t in range(0, ki, 2):
            for m in (0, 1):
                for gi, wt in enumerate(xw[m]):
                    nc.tensor.matmul(
                        pend[m]["i" if gi == 0 else "f"][:],
                        lhsT=wt[:, t:t + 2, :], rhs=act_sb["x8"][:, t:t + 2, :],
                        start=(t == 0), stop=False, perf_mode=DR)
        del xw[0], xw[1]

        for m in range(mt):
            ch = 2 if m < 2 else 1
            if m + 3 < mt:
                load_xw(m + 3, chunks=ch)
            if m + 2 < mt:
                for n, tag in (("hi", "w1"), ("hf", "w3"), ("cf", "w4")):
                    w_h[n, m + 2] = load_w(n, tag, m + 2, chunks=ch, bufs=3)
            # stage phase-1b bf16 data on the lightly used queues
            if 3 <= m < 7:
                c = (m - 3) * 4
                nc.gpsimd.dma_start(act_sb["xb"][:, c:c + 4, :], xTb[:, c:c + 4, :])
            elif 7 <= m < 11:
                c = (m - 7) * 4
                nc.gpsimd.dma_start(act_sb["hb"][:, c:c + 4, :], hTb[:, c:c + 4, :])
            elif m == 11:
                wg[0] = load_w("ic", "wxg", 0, eng=nc.scalar)
            elif m == 12:
                whc[0] = load_w("hc", "w6", 0, eng=nc.gpsimd)

            ps = pend.pop(m)
            ps_i, ps_f = ps["i"], ps["f"]
            accum(ps_i, "hi", w_h.pop(("hi", m)), "h", False, True)
            nc.scalar.activation(i_all[:, m, :], ps_i[:], Sig,
                                 bias=bias_sb[:, m, 0:1], scale=inv_s)
            accum(ps_f, "hf", w_h.pop(("hf", m)), "h", False, False)
            accum(ps_f, "cf", w_h.pop(("cf", m)), "c", False, True)
            f_tmp = tpool.tile([p, b], f32, tag="f_tmp", name="f_tmp")
            nc.scalar.activation(f_tmp[:], ps_f[:], Sig,
                                 bias=bias_sb[:, m, 1:2], scale=inv_s)
            c0_t = cpool.tile([p, b], bf16, tag="c0", name="c0_t")
            nc.gpsimd.dma_start(c0_t[:], c0Tb[:, m, :])
            nc.vector.tensor_mul(t1_all[:, m, :], f_tmp[:], c0_t[:])
            if m + 2 < mt:
                x_parts_if(m + 2)

        # ---- phase 1b: g gate (bf16) + new cell state ----
        for m in range(mt):
            if m + 1 < mt:
                if m + 1 > 0 and (m + 1) not in wg:
                    wg[m + 1] = load_w("ic", "wxg", m + 1, eng=nc.scalar)
                if (m + 1) not in whc:
                    whc[m + 1] = load_w("hc", "w6", m + 1, eng=nc.sync)

            ps_g = ppool.tile([p, b], f32, tag="ps", name=f"ps_g{m}")
            accum(ps_g, "ic", wg.pop(m), "x", True, False)
            accum(ps_g, "hc", whc.pop(m), "h", False, True)
            g_tmp = tpool.tile([p, b], f32, tag="g_tmp", name="g_tmp")
            nc.scalar.activation(g_tmp[:], ps_g[:], Tanh,
                                 bias=bias_sb[:, m, 2:3], scale=inv_s)
            v1 = tpool.tile([p, b], f32, tag="v1", name="v1")
            nc.vector.tensor_mul(v1[:], i_all[:, m, :], g_tmp[:])
            c1_m = c1b_sb[:, m, :]
            nc.vector.tensor_add(c1_m, v1[:], t1_all[:, m, :])
            nc.vector.tensor_copy(out=c1m_sb[:, m, :], in_=c1_m)
            nc.sync.dma_start(c1T[:, m, :], c1_m)

            if m == mt - 2:
                # prefetch phase-2 m=0 weights on the lightly-loaded gpsimd
                # queue so phase 2's first matmuls don't wait on sync.
                load_ow(0, eng=nc.gpsimd)

        # ---- phase 2: o gate + h1 ----
        for m in range(mt):
            if m + 1 < mt:
                load_ow(m + 1)
            w_io, w_ho, w_co = p2w.pop(m)

            ps_o = ppool.tile([p, b], f32, tag="ps", name=f"ps_o{m}")
            accum(ps_o, "io", w_io, "x", True, False)
            accum(ps_o, "ho", w_ho, "h", False, False)
            accum(ps_o, "co", w_co, c1m_sb, False, True)

            # tanh(c1) doesn't depend on this iteration's matmuls: issue it
            # first so the scalar engine works while the PE finishes.
            tc1 = tpool.tile([p, b], f32, tag="tc1", name="tc1")
            nc.scalar.activation(tc1[:], c1b_sb[:, m, :], Tanh)
            o_act = tpool.tile([p, b], bf16, tag="o_act", name="o_act")
            h1_t = tpool.tile([p, b], bf16, tag="h1", name="h1_t")
            if m == mt - 1:
                # halved epilogue: the first half's output DMA starts while
                # the second half is still in the activation engine.
                for lo, hi in ((0, b // 2), (b // 2, b)):
                    nc.scalar.activation(o_act[:, lo:hi], ps_o[:, lo:hi], Sig,
                                         bias=bias_sb[:, m, 3:4], scale=inv_s)
                    nc.vector.tensor_mul(h1_t[:, lo:hi], o_act[:, lo:hi],
                                         tc1[:, lo:hi])
            else:
                nc.scalar.activation(o_act[:], ps_o[:], Sig,
                                     bias=bias_sb[:, m, 3:4], scale=inv_s)
                nc.vector.tensor_mul(h1_t[:], o_act[:], tc1[:])

            # outputs spread across queues to avoid a drain backlog at the
            # end of the kernel.
            if m >= mt - 2:
                q = b // 4
                nc.scalar.dma_start(ogT[:, m, :q], o_act[:, :q])
                nc.sync.dma_start(ogT[:, m, q:2 * q], o_act[:, q:2 * q])
                nc.sync.dma_start(ogT[:, m, 2 * q:3 * q], o_act[:, 2 * q:3 * q])
                nc.scalar.dma_start(ogT[:, m, 3 * q:], o_act[:, 3 * q:])
                nc.scalar.dma_start(h1T[:, m, :q], h1_t[:, :q])
                nc.sync.dma_start(h1T[:, m, q:2 * q], h1_t[:, q:2 * q])
                nc.scalar.dma_start(h1T[:, m, 2 * q:3 * q], h1_t[:, 2 * q:3 * q])
                nc.sync.dma_start(h1T[:, m, 3 * q:], h1_t[:, 3 * q:])
            else:
                nc.scalar.dma_start(ogT[:, m, :], o_act[:])
                nc.gpsimd.dma_start(h1T[:, m, :b // 2], h1_t[:, :b // 2])
                nc.sync.dma_start(h1T[:, m, b // 2:], h1_t[:, b // 2:])

    nc.compile()
    return nc


_NC = None
_NC_KEY = None


def _get_nc():
    global _NC, _NC_KEY
    key = frozenset(FP8_SET)
    if _NC is None or _NC_KEY != key:
        _NC = _build(P, KI, KH, MT, B, key)
        _NC_KEY = key
    return _NC


# ---------------- host-side packing ----------------

def _pack_actT(a, dtype):
    """(b, d) -> (128, d//128, b) with [ki, ko, b] = a[b, ko*128+ki]."""
    b, d = a.shape
    return np.ascontiguousarray(
        a.T.reshape(d // P, P, b).transpose(1, 0, 2)
    ).astype(dtype, copy=False)


def _pack_w(W, dtype):
    """(H, K) -> (H//128, 128, K//128, 128) with [mt, ki, ko, m] = s*W[mt*128+m, ko*128+ki]."""
    H, K = W.shape
    return np.ascontiguousarray(
        (W.reshape(H // P, P, K // P, P) * W_SCALE)
        .transpose(0, 3, 2, 1).astype(dtype)
    )


def _unpack_out(o):
    """(128, mt, b) [p, m, b] -> (b, mt*128) fp32."""
    p, m, b = o.shape
    return np.ascontiguousarray(
        o.transpose(2, 1, 0).reshape(b, m * p).astype(np.float32))


def kernel(x, h0, c0,
           W_ii, b_ii, W_hi, b_hi, W_if_, b_if_, W_hf, b_hf, W_cf, b_cf,
           W_ic, b_ic, W_hc, b_hc, W_io, b_io, W_ho, b_ho, W_co, b_co,
           _trace=False):
    from concourse.bass_utils import run_bass_kernel_spmd

    nc = _get_nc()

    x = np.asarray(x, dtype=np.float32)
    h0 = np.asarray(h0, dtype=np.float32)
    c0 = np.asarray(c0, dtype=np.float32)
    Ws = dict(zip(W_NAMES, [W_ii, W_hi, W_if_, W_hf, W_cf,
                            W_ic, W_hc, W_io, W_ho, W_co]))
    Ws = {n: np.asarray(a, dtype=np.float32) for n, a in Ws.items()}
    (b_ii, b_hi, b_if_, b_hf, b_cf, b_ic, b_hc, b_io, b_ho, b_co) = [
        np.asarray(a, dtype=np.float32)
        for a in (b_ii, b_hi, b_if_, b_hf, b_cf, b_ic, b_hc, b_io, b_ho, b_co)
    ]

    # combined per-gate biases, packed [p, mt, gate]
    bias = np.stack(
        [
            (b_ii + b_hi).reshape(MT, P).T,
            (b_if_ + b_hf + b_cf).reshape(MT, P).T,
            (b_ic + b_hc).reshape(MT, P).T,
            (b_io + b_ho + b_co).reshape(MT, P).T,
        ],
        axis=2,
    ).astype(np.float32)
    w_packed = {
        f"w_{n}": _pack_w(W, F8 if n in FP8_SET else BF)
        for n, W in Ws.items()
    }

    in_maps = []
    for core in range(N_CORES):
        s = slice(core * B, (core + 1) * B)
        m = {
            "xTb": _pack_actT(x[s], BF),
            "hTb": _pack_actT(h0[s], BF),
            "xT8": _pack_actT(x[s], F8),
            "hT8": _pack_actT(h0[s], F8),
            "c0Tb": _pack_actT(c0[s], BF),
            "bias": bias,
        }
        if "cf" in FP8_SET:
            m["cT8"] = _pack_actT(c0[s], F8)
        else:
            m["cTb"] = _pack_actT(c0[s], BF)
        m.update(w_packed)
        in_maps.append(m)

    res = run_bass_kernel_spmd(nc, in_maps, list(range(N_CORES)), trace=_trace)

    o_g = np.empty((BATCH, HID), np.float32)
    h1 = np.empty((BATCH, HID), np.float32)
    c1 = np.empty((BATCH, HID), np.float32)
    for core in range(N_CORES):
        s = slice(core * B, (core + 1) * B)
        o_g[s] = _unpack_out(res.results[core]["ogT"])
        h1[s] = _unpack_out(res.results[core]["h1T"])
        c1[s] = _unpack_out(res.results[core]["c1T"])
    out = (o_g, h1, c1)
    if _trace:
        return out, res
    return out


# revision 33
# speedup vs baseline: 1.0135x; 1.0135x over previous
"""Trainium2 Bass kernel for the nn_LSTMCell problem.

Strategy: data-parallel over the batch dim (4096 -> 8 cores x 512), weights
replicated. All on-chip compute happens in "transposed" orientation
(hidden on PSUM partitions, batch on the free dim) so every matmul operand
can be DMA'd in its natural, contiguous layout:

    gate.T[h, b] = sum_k W.T[k, h] * act.T[k, b]

Matmuls run in fp8e4 (DoubleRow, 2 k-tiles per instruction, 2x bf16
instruction throughput) except the g-gate (tanh path: no sigmoid
attenuation, dominant error contributor), which stays bf16. All weights
(both dtypes) are pre-scaled x256 on the host so the fp8 ones sit in
e4m3's normal range (raw |W|<=0.023 would quantize as subnormals with
~20% relative error); the 1/256 is folded into the scalar-engine
activation instruction: out = func(psum/256 + bias). PSUM accumulation is
fp32; elementwise math is fp32; outputs are written bf16 and upcast on
the host.

The DMA path takes ~10us to move its first bytes and early bandwidth is
the ramp bottleneck, so early bytes are minimized: only bf16 activations
are loaded (x, h) and their fp8 copies are derived on-device by DVE
casts; phase 1 is software-pipelined so the i/f x-side accums of m=0/1
(x + four small fp8 slabs, ~3.75MB) are the only prologue dependencies,
with the bf16 g-gate work deferred into the per-m iterations.

Per core:
  phase 1 iteration m: h-side accums + cf for m, activations,
           c1 = f*c0 + i*tanh(g) (fp32 in SBUF, bf16 out), c1 cast to
           fp8, then i/f x-side accums of m+2.
  phase 2: per h-tile: o gate fp8 matmuls (incl. W_co @ c1.T),
           o = sigmoid(...), h1 = o * tanh(c1), DMA out (bf16).
"""

import numpy as np
import ml_dtypes
from contextlib import ExitStack

BF = ml_dtypes.bfloat16
F8 = ml_dtypes.float8_e4m3   # TRN FP8_EXP4 (max +-240)
W_SCALE = 256.0              # weights pre-scaled into e4m3 normal range

N_CORES = 8
P = 128          # partition dim / k-tile size / m-tile size
BATCH = 4096
IN_DIM = 2048
HID = 2048
B = BATCH // N_CORES          # 512, batch per core = matmul free dim
KI = IN_DIM // P              # 16, k-tiles for x contraction
KH = HID // P                 # 16, k-tiles for h/c contraction
MT = HID // P                 # 16, output h-tiles

W_NAMES = ["ii", "hi", "if_", "hf", "cf", "ic", "hc", "io", "ho", "co"]
X_NAMES = ("ii", "if_", "ic", "io")   # weights contracting over x
# matmuls run in fp8 DoubleRow except the g-gate (tanh path)
FP8_SET = frozenset(W_NAMES) - {"ic", "hc"}


def _build(p, ki, kh, mt, b, fp8_set):
    import concourse.tile as tile
    from concourse import bacc, mybir

    bf16, f32, f8 = mybir.dt.bfloat16, mybir.dt.float32, mybir.dt.float8e4
    Sig = mybir.ActivationFunctionType.Sigmoid
    Tanh = mybir.ActivationFunctionType.Tanh
    DR = mybir.MatmulPerfMode.DoubleRow
    inv_s = 1.0 / W_SCALE

    nc = bacc.Bacc(
        "TRN2",
        target_bir_lowering=False,
        debug=False,
        num_devices=N_CORES,
    )

    def wdt(n):
        return f8 if n in fp8_set else bf16

    need8 = {"x": any(n in fp8_set for n in X_NAMES),
             "h": any(n in fp8_set for n in ("hi", "hf", "hc", "ho"))}

    xTb = nc.dram_tensor("xTb", [p, ki, b], bf16, kind="ExternalInput").ap()
    hTb = nc.dram_tensor("hTb", [p, kh, b], bf16, kind="ExternalInput").ap()
    xT8 = nc.dram_tensor("xT8", [p, ki, b], f8, kind="ExternalInput").ap() if need8["x"] else None
    hT8 = nc.dram_tensor("hT8", [p, kh, b], f8, kind="ExternalInput").ap() if need8["h"] else None
    if "cf" in fp8_set:
        cT = nc.dram_tensor("cT8", [p, kh, b], f8, kind="ExternalInput").ap()
    else:
        cT = nc.dram_tensor("cTb", [p, kh, b], bf16, kind="ExternalInput").ap()
    c0Tb = nc.dram_tensor("c0Tb", [p, mt, b], bf16, kind="ExternalInput").ap()
    bias = nc.dram_tensor("bias", [p, mt, 4], f32, kind="ExternalInput").ap()
    w = {
        n: nc.dram_tensor(
            f"w_{n}", [mt, p, (ki if n in X_NAMES else kh), p],
            wdt(n), kind="ExternalInput",
        ).ap()
        for n in W_NAMES
    }
    ogT = nc.dram_tensor("ogT", [p, mt, b], bf16, kind="ExternalOutput").ap()
    h1T = nc.dram_tensor("h1T", [p, mt, b], bf16, kind="ExternalOutput").ap()
    c1T = nc.dram_tensor("c1T", [p, mt, b], bf16, kind="ExternalOutput").ap()

    with tile.TileContext(nc) as tc, ExitStack() as ctx:
        acts = ctx.enter_context(tc.tile_pool(name="acts", bufs=1))
        wpool = ctx.enter_context(tc.tile_pool(name="w", bufs=2))
        cpool = ctx.enter_context(tc.tile_pool(name="c0", bufs=2))
        tpool = ctx.enter_context(tc.tile_pool(name="temps", bufs=2))
        ppool = ctx.enter_context(tc.tile_pool(name="psum", bufs=8, space="PSUM"))

        act_sb = {}
        act_sb["xb"] = acts.tile([p, ki, b], bf16, tag="xb", name="xb_sb")
        act_sb["hb"] = acts.tile([p, kh, b], bf16, tag="hb", name="hb_sb")
        ckey = "c8" if "cf" in fp8_set else "cb"
        act_sb[ckey] = acts.tile([p, kh, b], f8 if ckey == "c8" else bf16,
                                 tag="cc", name="c_sb")
        if need8["x"]:
            act_sb["x8"] = acts.tile([p, ki, b], f8, tag="x8", name="x8_sb")
        if need8["h"]:
            act_sb["h8"] = acts.tile([p, kh, b], f8, tag="h8", name="h8_sb")

        bias_sb = acts.tile([p, mt, 4], f32, tag="bias", name="bias_sb")
        # staged per-m results of phase 1a, consumed by phase 1b
        i_all = acts.tile([p, mt, b], bf16, tag="i_all", name="i_all")
        t1_all = acts.tile([p, mt, b], bf16, tag="t1_all", name="t1_all")
        c1b_sb = acts.tile([p, mt, b], bf16, tag="c1b", name="c1b_sb")
        c1m_dt = f8 if "co" in fp8_set else bf16
        c1m_sb = acts.tile([p, mt, b], c1m_dt, tag="c1m", name="c1m_sb")

        def load_w(name, tag, m, chunks=1, eng=None, bufs=2):
            nk = w[name].shape[2]
            t = wpool.tile([p, nk, p], wdt(name), tag=tag, name=f"w_{tag}_{m}",
                           bufs=bufs)
            step = max(1, nk // chunks)
            for c in range(0, nk, step):
                (eng or nc.sync).dma_start(t[:, c:c + step], w[name][m, :, c:c + step])
            return t

        def accum(ps, name, w_t, act_key, first, last):
            fp8 = name in fp8_set
            if isinstance(act_key, str):
                a = act_sb[act_key + ("8" if fp8 else "b")]
            else:
                a = act_key
            nk = w_t.shape[1]
            if fp8:
                for t in range(0, nk, 2):
                    nc.tensor.matmul(
                        ps[:], lhsT=w_t[:, t:t + 2, :], rhs=a[:, t:t + 2, :],
                        start=(first and t == 0), stop=(last and t == nk - 2),
                        perf_mode=DR,
                    )
            else:
                for t in range(nk):
                    nc.tensor.matmul(
                        ps[:], lhsT=w_t[:, t], rhs=a[:, t],
                        start=(first and t == 0), stop=(last and t == nk - 1),
                    )

        # ---- phase 1a: i/f gates -- all-fp8 data, minimal ramp bytes ----
        xw = {}       # m -> (w_ii, w_if) tiles
        wg = {}       # m -> w_ic tile (bf16, phase 1b)
        whc = {}      # m -> w_hc tile (bf16, phase 1b)
        pend = {}     # m -> {"i"/"f": psum tile}
        p2w = {}      # m -> (w_io, w_ho, w_co) tiles

        def load_xw(m, eng=None, chunks=1):
            xw[m] = (load_w("ii", "wxi", m, chunks, eng, bufs=3),
                     load_w("if_", "wxf", m, chunks, eng, bufs=3))

        def load_ow(m, eng=None):
            p2w[m] = (load_w("io", "po", m, eng=eng),
                      load_w("ho", "ph", m, eng=eng),
                      load_w("co", "pc", m, eng=eng))

        def x_parts_if(m):
            ps = pend.setdefault(m, {})
            ps["i"] = ppool.tile([p, b], f32, tag="ps", name=f"ps_i{m}")
            accum(ps["i"], "ii", xw[m][0], "x", True, False)
            ps["f"] = ppool.tile([p, b], f32, tag="ps", name=f"ps_f{m}")
            accum(ps["f"], "if_", xw[m][1], "x", True, False)
            del xw[m]

        # prologue: tiny pokes spin up all three DMA queues, then
        # first-use-ordered fp8 loads. The bf16 g-gate data (xb/hb/w_ic/
        # w_hc, ~5MB) is staged lazily through phase 1a's iterations --
        # it isn't needed until phase 1b, ~90us in.
        nc.gpsimd.dma_start(bias_sb[:, 0:8], bias[:, 0:8])
        nc.sync.dma_start(bias_sb[:, 8:12], bias[:, 8:12])
        nc.scalar.dma_start(bias_sb[:, 12:16], bias[:, 12:16])
        for lo, hi in ((0, 2), (2, 4), (4, 8), (8, 12), (12, ki)):
            nc.gpsimd.dma_start(act_sb["x8"][:, lo:hi, :], xT8[:, lo:hi, :])
        # PE warmup: dummy matmuls on (garbage) SBUF keep the PE busy while
        # the DMA path spins up (~10us to first bytes), so the HAM clock
        # gate reaches 8/8 before the first real matmul instead of running
        # the whole ramp at half clock. Results land in a PSUM bank that is
        # cleared on its next start=True use.
        dum_w = acts.tile([p, 2, p], f8, tag="dum_w", name="dum_w")
        dum_r = acts.tile([p, 2, b], f8, tag="dum_r", name="dum_r")
        nc.vector.memset(dum_w[:], 0)
        nc.vector.memset(dum_r[:], 0)
        warm_ps = ppool.tile([p, b], f32, tag="ps", name="warm_ps")
        for _ in range(26):
            nc.tensor.matmul(warm_ps[:], lhsT=dum_w[:], rhs=dum_r[:],
                             start=True, stop=True, perf_mode=DR)
        load_xw(0, eng=nc.scalar, chunks=2)
        load_xw(1, eng=nc.scalar)
        w_h = {}
        w_h["hi", 0] = load_w("hi", "w1", 0, chunks=2, eng=nc.scalar, bufs=3)
        w_h["hf", 0] = load_w("hf", "w3", 0, chunks=2, eng=nc.scalar, bufs=3)
        for c in range(0, kh, 4):
            nc.gpsimd.dma_start(act_sb["h8"][:, c:c + 4, :], hT8[:, c:c + 4, :])
        w_h["cf", 0] = load_w("cf", "w4", 0, eng=nc.sync, bufs=3)
        for c in range(0, kh, 4):
            nc.gpsimd.dma_start(act_sb[ckey][:, c:c + 4, :], cT[:, c:c + 4, :])
        for n, tag in (("hi", "w1"), ("hf", "w3"), ("cf", "w4")):
            w_h[n, 1] = load_w(n, tag, 1, chunks=2, eng=nc.sync, bufs=3)
        load_xw(2)
        for m in (0, 1):
            ps = pend.setdefault(m, {})
            ps["i"] = ppool.tile([p, b], f32, tag="ps", name=f"ps_i{m}")
            ps["f"] = ppool.tile([p, b], f32, tag="ps", name=f"ps_f{m}")
        for t in range(0, ki, 2):
            for m in (0, 1):
                for gi, wt in enumerate(xw[m]):
                    nc.tensor.matmul(
                        pend[m]["i" if gi == 0 else "f"][:],
                        lhsT=wt[:, t:t + 2, :], rhs=act_sb["x8"][:, t:t + 2, :],
                        start=(t == 0), stop=False, perf_mode=DR)
        del xw[0], xw[1]

        for m in range(mt):
            ch = 2 if m < 2 else 1
            if m + 3 < mt:
                load_xw(m + 3, chunks=ch)
            if m + 2 < mt:
                for n, tag in (("hi", "w1"), ("hf", "w3"), ("cf", "w4")):
                    w_h[n, m + 2] = load_w(n, tag, m + 2, chunks=ch, bufs=3)
            # stage phase-1b bf16 data on the lightly used queues
            if 3 <= m < 7:
                c = (m - 3) * 4
                nc.gpsimd.dma_start(act_sb["xb"][:, c:c + 4, :], xTb[:, c:c + 4, :])
            elif 7 <= m < 11:
                c = (m - 7) * 4
                nc.gpsimd.dma_start(act_sb["hb"][:, c:c + 4, :], hTb[:, c:c + 4, :])
            elif m == 11:
                wg[0] = load_w("ic", "wxg", 0, eng=nc.scalar)
            elif m == 12:
                whc[0] = load_w("hc", "w6", 0, eng=nc.gpsimd)

            ps = pend.pop(m)
            ps_i, ps_f = ps["i"], ps["f"]
            accum(ps_i, "hi", w_h.pop(("hi", m)), "h", False, True)
            nc.scalar.activation(i_all[:, m, :], ps_i[:], Sig,
                                 bias=bias_sb[:, m, 0:1], scale=inv_s)
            accum(ps_f, "hf", w_h.pop(("hf", m)), "h", False, False)
            accum(ps_f, "cf", w_h.pop(("cf", m)), "c", False, True)
            f_tmp = tpool.tile([p, b], f32, tag="f_tmp", name="f_tmp")
            nc.scalar.activation(f_tmp[:], ps_f[:], Sig,
                                 bias=bias_sb[:, m, 1:2], scale=inv_s)
            c0_t = cpool.tile([p, b], bf16, tag="c0", name="c0_t")
            nc.gpsimd.dma_start(c0_t[:], c0Tb[:, m, :])
            nc.vector.tensor_mul(t1_all[:, m, :], f_tmp[:], c0_t[:])
            if m + 2 < mt:
                x_parts_if(m + 2)

        # ---- phase 1b: g gate (bf16) + new cell state ----
        for m in range(mt):
            if m + 1 < mt:
                if m + 1 > 0 and (m + 1) not in wg:
                    wg[m + 1] = load_w("ic", "wxg", m + 1, eng=nc.scalar)
                if (m + 1) not in whc:
                    whc[m + 1] = load_w("hc", "w6", m + 1, eng=nc.sync)

            ps_g = ppool.tile([p, b], f32, tag="ps", name=f"ps_g{m}")
            accum(ps_g, "ic", wg.pop(m), "x", True, False)
            accum(ps_g, "hc", whc.pop(m), "h", False, True)
            g_tmp = tpool.tile([p, b], f32, tag="g_tmp", name="g_tmp")
            nc.scalar.activation(g_tmp[:], ps_g[:], Tanh,
                                 bias=bias_sb[:, m, 2:3], scale=inv_s)
            v1 = tpool.tile([p, b], f32, tag="v1", name="v1")
            nc.vector.tensor_mul(v1[:], i_all[:, m, :], g_tmp[:])
            c1_m = c1b_sb[:, m, :]
            nc.vector.tensor_add(c1_m, v1[:], t1_all[:, m, :])
            nc.vector.tensor_copy(out=c1m_sb[:, m, :], in_=c1_m)
            nc.sync.dma_start(c1T[:, m, :], c1_m)

            if m == mt - 2:
                # prefetch phase-2 m=0 weights on the lightly-loaded gpsimd
                # queue so phase 2's first matmuls don't wait on sync.
                load_ow(0, eng=nc.gpsimd)

        # ---- phase 2: o gate + h1 ----
        for m in range(mt):
            if m + 1 < mt:
                load_ow(m + 1)
            w_io, w_ho, w_co = p2w.pop(m)

            ps_o = ppool.tile([p, b], f32, tag="ps", name=f"ps_o{m}")
            accum(ps_o, "io", w_io, "x", True, False)
            accum(ps_o, "ho", w_ho, "h", False, False)
            accum(ps_o, "co", w_co, c1m_sb, False, True)

            # tanh(c1) doesn't depend on this iteration's matmuls: issue it
            # first so the scalar engine works while the PE finishes.
            tc1 = tpool.tile([p, b], f32, tag="tc1", name="tc1")
            nc.scalar.activation(tc1[:], c1b_sb[:, m, :], Tanh)
            o_act = tpool.tile([p, b], bf16, tag="o_act", name="o_act")
            h1_t = tpool.tile([p, b], bf16, tag="h1", name="h1_t")
            if m == mt - 1:
                # halved epilogue: the first half's output DMA starts while
                # the second half is still in the activation engine.
                for lo, hi in ((0, b // 2), (b // 2, b)):
                    nc.scalar.activation(o_act[:, lo:hi], ps_o[:, lo:hi], Sig,
                                         bias=bias_sb[:, m, 3:4], scale=inv_s)
                    nc.vector.tensor_mul(h1_t[:, lo:hi], o_act[:, lo:hi],
                                         tc1[:, lo:hi])
            else:
                nc.scalar.activation(o_act[:], ps_o[:], Sig,
                                     bias=bias_sb[:, m, 3:4], scale=inv_s)
                nc.vector.tensor_mul(h1_t[:], o_act[:], tc1[:])

            # outputs spread across queues to avoid a drain backlog at the
            # end of the kernel.
            if m >= mt - 2:
                q = b // 4
                nc.scalar.dma_start(ogT[:, m, :q], o_act[:, :q])
                nc.sync.dma_start(ogT[:, m, q:2 * q], o_act[:, q:2 * q])
                nc.sync.dma_start(ogT[:, m, 2 * q:3 * q], o_act[:, 2 * q:3 * q])
                nc.scalar.dma_start(ogT[:, m, 3 * q:], o_act[:, 3 * q:])
                nc.scalar.dma_start(h1T[:, m, :q], h1_t[:, :q])
                nc.sync.dma_start(h1T[:, m, q:2 * q], h1_t[:, q:2 * q])
                nc.scalar.dma_start(h1T[:, m, 2 * q:3 * q], h1_t[:, 2 * q:3 * q])
                nc.sync.dma_start(h1T[:, m, 3 * q:], h1_t[:, 3 * q:])
            else:
                nc.scalar.dma_start(ogT[:, m, :], o_act[:])
                nc.gpsimd.dma_start(h1T[:, m, :b // 2], h1_t[:, :b // 2])
                nc.sync.dma_start(h1T[:, m, b // 2:], h1_t[:, b // 2:])

    nc.compile()
    return nc


_NC = None
_NC_KEY = None


def _get_nc():
    global _NC, _NC_KEY
    key = frozenset(FP8_SET)
    if _NC is None or _NC_KEY != key:
        _NC = _build(P, KI, KH, MT, B, key)
        _NC_KEY = key
    return _NC


# ---------------- host-side packing ----------------

def _pack_actT(a, dtype):
    """(b, d) -> (128, d//128, b) with [ki, ko, b] = a[b, ko*128+ki]."""
    b, d = a.shape
    return np.ascontiguousarray(
        a.T.reshape(d // P, P, b).transpose(1, 0, 2)
    ).astype(dtype, copy=False)


def _pack_w(W, dtype):
    """(H, K) -> (H//128, 128, K//128, 128) with [mt, ki, ko, m] = s*W[mt*128+m, ko*128+ki]."""
    H, K = W.shape
    return np.ascontiguousarray(
        (W.reshape(H // P, P, K // P, P) * W_SCALE)
        .transpose(0, 3, 2, 1).astype(dtype)
    )


def _unpack_out(o):
    """(128, mt, b) [p, m, b] -> (b, mt*128) fp32."""
    p, m, b = o.shape
    return np.ascontiguousarray(
        o.transpose(2, 1, 0).reshape(b, m * p).astype(np.float32))


def kernel(x, h0, c0,
           W_ii, b_ii, W_hi, b_hi, W_if_, b_if_, W_hf, b_hf, W_cf, b_cf,
           W_ic, b_ic, W_hc, b_hc, W_io, b_io, W_ho, b_ho, W_co, b_co,
           _trace=False):
    from concourse.bass_utils import run_bass_kernel_spmd

    nc = _get_nc()

    x = np.asarray(x, dtype=np.float32)
    h0 = np.asarray(h0, dtype=np.float32)
    c0 = np.asarray(c0, dtype=np.float32)
    Ws = dict(zip(W_NAMES, [W_ii, W_hi, W_if_, W_hf, W_cf,
                            W_ic, W_hc, W_io, W_ho, W_co]))
    Ws = {n: np.asarray(a, dtype=np.float32) for n, a in Ws.items()}
    (b_ii, b_hi, b_if_, b_hf, b_cf, b_ic, b_hc, b_io, b_ho, b_co) = [
        np.asarray(a, dtype=np.float32)
        for a in (b_ii, b_hi, b_if_, b_hf, b_cf, b_ic, b_hc, b_io, b_ho, b_co)
    ]

    # combined per-gate biases, packed [p, mt, gate]
    bias = np.stack(
        [
            (b_ii + b_hi).reshape(MT, P).T,
            (b_if_ + b_hf + b_cf).reshape(MT, P).T,
            (b_ic + b_hc).reshape(MT, P).T,
            (b_io + b_ho + b_co).reshape(MT, P).T,
        ],
        axis=2,
    ).astype(np.float32)
    w_packed = {
        f"w_{n}": _pack_w(W, F8 if n in FP8_SET else BF)
        for n, W in Ws.items()
    }

    in_maps = []
    for core in range(N_CORES):
        s = slice(core * B, (core + 1) * B)
        m = {
            "xTb": _pack_actT(x[s], BF),
            "hTb": _pack_actT(h0[s], BF),
            "xT8": _pack_actT(x[s], F8),
            "hT8": _pack_actT(h0[s], F8),
            "c0Tb": _pack_actT(c0[s], BF),
            "bias": bias,
        }
        if "cf" in FP8_SET:
            m["cT8"] = _pack_actT(c0[s], F8)
        else:
            m["cTb"] = _pack_actT(c0[s], BF)
        m.update(w_packed)
        in_maps.append(m)

    res = run_bass_kernel_spmd(nc, in_maps, list(range(N_CORES)), trace=_trace)

    o_g = np.empty((BATCH, HID), np.float32)
    h1 = np.empty((BATCH, HID), np.float32)
    c1 = np.empty((BATCH, HID), np.float32)
    for core in range(N_CORES):
        s = slice(core * B, (core + 1) * B)
        o_g[s] = _unpack_out(res.results[core]["ogT"])
        h1[s] = _unpack_out(res.results[core]["h1T"])
        c1[s] = _unpack_out(res.results[core]["c1T"])
    out = (o_g, h1, c1)
    if _trace:
        return out, res
    return out
